# revision 2
# baseline (speedup 1.0000x reference)
"""Trainium2 Bass kernel for nn_BaseQVLayer (GNN message passing).

Reference computation (single device):
    xp = x @ Wx + bx                      # [Nx, E]
    yp = y @ Wy + by                      # [Ny, E]
    A_ = xp @ yp.T                        # [Nx, Ny]
    A  = 2*A_ / (||xp_i||^2 + ||yp_j||^2) # Dice-style normalization
    gwf = A.T @ xp                        # [Ny, E]
    out = relu(gwf @ Wg + bg)             # [Ny, E]

Distribution: column-parallel over Ny (8 shards of 1024 y-rows, one per
NeuronCore).  Each core needs the *full* xp in two layouts (normal for the
gwf contraction, transposed for the A matmul) plus its own ypT shard.  The
xp/xpT/|xp|^2 pieces are computed on each core for its own Nx shard only and
exchanged with a single packed AllGather; everything downstream is local to
the core, so there is no AllReduce at all.  Matmuls run in float32r
(full-rate 4-byte PE mode, ~1e-4 relative rounding).

Per-core phases:
  1. project own shards: xp_shard, xpT_shard, Dcol_shard, ypT_shard, Drow
  2. packed AllGather of (xp, xpT, Dcol) shards
  3. two ny-sub passes (512 each): stream xpT/xp tiles, compute A tiles,
     normalize with 2/(Dcol+Drow) via fast reciprocal, accumulate
     gwfT = xp.T @ A in PSUM, then fused ReLU-MLP and store outT.

kernel(**inputs) takes full unsharded inputs and returns the full output.
"""

import sys

if "/opt/trn_rl_repo" not in sys.path:
    sys.path.insert(0, "/opt/trn_rl_repo")

import numpy as np

NCORES = 8
NX, NY = 8192, 8192
FX, FY = 1024, 1024
EMB, EMB_OUT = 512, 512

P = 128
KT = FX // P           # 8   k-tiles over feature dim
ME = EMB // P          # 4   emb tiles
NSH = NX // NCORES     # 1024 rows per shard
TSH = NSH // P         # 8   nx tiles per shard
TALL = NX // P         # 64  nx tiles total
NYSUB = 512            # ny columns per pass
NSUBS = NSH // NYSUB   # 2   passes

XP_ELEMS = P * TSH * EMB          # 524288
XPT_ELEMS = P * ME * NSH          # 524288
DCOL_ELEMS = P * TSH              # 1024
SH_ELEMS = XP_ELEMS + XPT_ELEMS + DCOL_ELEMS  # 1049600

_CACHE = {}


def _build_nc():
    import concourse.bass as bass
    from concourse import bacc
    import concourse.mybir as mybir
    import concourse.tile as tile

    F32 = mybir.dt.float32
    F32R = mybir.dt.float32r
    ALU = mybir.AluOpType
    ACTF = mybir.ActivationFunctionType

    nc = bacc.Bacc("TRN2", target_bir_lowering=False, debug=False,
                   num_devices=NCORES)

    xT = nc.dram_tensor("xT", [FX, NSH], F32, kind="ExternalInput")
    yT = nc.dram_tensor("yT", [FY, NSH], F32, kind="ExternalInput")
    Wx = nc.dram_tensor("Wx", [FX, EMB], F32, kind="ExternalInput")
    Wy = nc.dram_tensor("Wy", [FY, EMB], F32, kind="ExternalInput")
    Wg = nc.dram_tensor("Wg", [EMB, EMB_OUT], F32, kind="ExternalInput")
    bx_bc = nc.dram_tensor("bx_bc", [P, EMB], F32, kind="ExternalInput")
    bxp = nc.dram_tensor("bxp", [P, ME], F32, kind="ExternalInput")
    byp = nc.dram_tensor("byp", [P, ME], F32, kind="ExternalInput")
    bgp = nc.dram_tensor("bgp", [P, EMB_OUT // P], F32, kind="ExternalInput")
    ones = nc.dram_tensor("ones", [P, P], F32, kind="ExternalInput")
    outT = nc.dram_tensor("outT", [EMB_OUT, NSH], F32, kind="ExternalOutput")

    with tile.TileContext(nc) as tc:
        with (
            tc.tile_pool(name="perm", bufs=1) as perm,
            tc.tile_pool(name="psA", bufs=3, space="PSUM") as psA,
            tc.tile_pool(name="dramp", bufs=1, space="DRAM") as dramp,
        ):
            # ---- permanent tiles ----
            ypT_sb = perm.tile([P, ME, NSH], F32R)
            drow_sb = perm.tile([P, NSH], F32)
            dcol_sb = perm.tile([P, TALL], F32)
            Wg_sb = perm.tile([P, ME, EMB_OUT], F32R)
            bgp_sb = perm.tile([P, EMB_OUT // P], F32)
            nc.sync.dma_start(
                Wg_sb[:], Wg.ap().rearrange("(kt p) n -> p kt n", p=P).bitcast(F32R))
            nc.sync.dma_start(bgp_sb[:], bgp.ap())

            ag_in = dramp.tile([SH_ELEMS], F32)
            ag_out = dramp.tile([NCORES * SH_ELEMS], F32, addr_space="Shared")

            # ================= phase 1: own-shard projections =================
            with (
                tc.tile_pool(name="wpool", bufs=1) as wpool,
                tc.tile_pool(name="shard", bufs=1) as shard,
                tc.tile_pool(name="scr", bufs=2) as scr,
                tc.tile_pool(name="ph1ps", bufs=2, space="PSUM") as ph1ps,
            ):
                xT_sb = wpool.tile([P, KT, NSH], F32R)
                yT_sb = wpool.tile([P, KT, NSH], F32R)
                Wx_sb = wpool.tile([P, KT, EMB], F32R)
                Wy_sb = wpool.tile([P, KT, EMB], F32R)
                bx_bc_sb = wpool.tile([P, EMB], F32)
                bxp_sb = wpool.tile([P, ME], F32)
                byp_sb = wpool.tile([P, ME], F32)
                ones_sb = wpool.tile([P, P], F32R)
                nc.sync.dma_start(
                    xT_sb[:], xT.ap().rearrange("(kt p) n -> p kt n", p=P).bitcast(F32R))
                nc.sync.dma_start(
                    Wx_sb[:], Wx.ap().rearrange("(kt p) n -> p kt n", p=P).bitcast(F32R))
                nc.sync.dma_start(
                    yT_sb[:], yT.ap().rearrange("(kt p) n -> p kt n", p=P).bitcast(F32R))
                nc.sync.dma_start(
                    Wy_sb[:], Wy.ap().rearrange("(kt p) n -> p kt n", p=P).bitcast(F32R))
                nc.sync.dma_start(bx_bc_sb[:], bx_bc.ap())
                nc.sync.dma_start(bxp_sb[:], bxp.ap())
                nc.sync.dma_start(byp_sb[:], byp.ap())
                nc.sync.dma_start(ones_sb[:], ones.ap().bitcast(F32R))

                xp_sb = shard.tile([P, TSH, EMB], F32R)
                xpT_sb = shard.tile([P, ME, NSH], F32R)
                dcol_own = shard.tile([P, TSH], F32)

                # xp shard: [128, m, 512] , nx on partitions
                for m in range(TSH):
                    ps = psA.tile([P, EMB], mybir.dt.float32, tag="mm", name="ps_xp")
                    for k in range(KT):
                        nc.tensor.matmul(
                            ps[:], xT_sb[:, k, m * P:(m + 1) * P], Wx_sb[:, k, :],
                            start=(k == 0), stop=(k == KT - 1))
                    nc.vector.tensor_tensor(
                        xp_sb[:, m, :], ps[:], bx_bc_sb[:], ALU.add)
                    sq = scr.tile([P, EMB], F32, tag="sq", name="sq")
                    nc.scalar.activation(
                        sq[:], xp_sb[:, m, :].bitcast(F32), ACTF.Square,
                        scale=1.0, accum_out=dcol_own[:, m:m + 1])

                # xpT shard: [128, me, 1024], emb on partitions
                for m in range(ME):
                    for nb in range(NSH // 512):
                        ps = psA.tile([P, 512], mybir.dt.float32, tag="mm",
                                      name="ps_xpt")
                        for k in range(KT):
                            nc.tensor.matmul(
                                ps[:], Wx_sb[:, k, m * P:(m + 1) * P],
                                xT_sb[:, k, nb * 512:(nb + 1) * 512],
                                start=(k == 0), stop=(k == KT - 1))
                        nc.scalar.activation(
                            xpT_sb[:, m, nb * 512:(nb + 1) * 512], ps[:],
                            ACTF.Identity, bias=bxp_sb[:, m:m + 1], scale=1.0)

                # pack + AllGather
                ap = ag_in[:]
                nc.sync.dma_start(
                    ap[0:XP_ELEMS]
                    .rearrange("(p m e) -> p m e", p=P, m=TSH).bitcast(F32R),
                    xp_sb[:])
                nc.sync.dma_start(
                    ap[XP_ELEMS:XP_ELEMS + XPT_ELEMS]
                    .rearrange("(p m n) -> p m n", p=P, m=ME).bitcast(F32R),
                    xpT_sb[:])
                nc.sync.dma_start(
                    ap[XP_ELEMS + XPT_ELEMS:SH_ELEMS]
                    .rearrange("(p m) -> p m", p=P),
                    dcol_own[:])
                nc.gpsimd.collective_compute(
                    "AllGather", ALU.bypass,
                    replica_groups=[list(range(NCORES))],
                    ins=[ag_in[:].opt()],
                    outs=[ag_out[:].opt()],
                )

                # ypT shard (overlaps the AllGather)
                for m in range(ME):
                    for nb in range(NSH // 512):
                        ps = psA.tile([P, 512], mybir.dt.float32, tag="mm",
                                      name="ps_ypt")
                        for k in range(KT):
                            nc.tensor.matmul(
                                ps[:], Wy_sb[:, k, m * P:(m + 1) * P],
                                yT_sb[:, k, nb * 512:(nb + 1) * 512],
                                start=(k == 0), stop=(k == KT - 1))
                        nc.scalar.activation(
                            ypT_sb[:, m, nb * 512:(nb + 1) * 512], ps[:],
                            ACTF.Identity, bias=byp_sb[:, m:m + 1], scale=1.0)

                # Drow = colsum(ypT^2) broadcast to all partitions via ones-matmul
                drow_ps = [
                    ph1ps.tile([P, 512], mybir.dt.float32, tag="drow",
                               name=f"drow_ps{nb}")
                    for nb in range(NSH // 512)
                ]
                for m in range(ME):
                    sqd = scr.tile([P, NSH], F32R, tag="sqd", name="sqd")
                    nc.vector.tensor_tensor(
                        sqd[:], ypT_sb[:, m, :].bitcast(F32),
                        ypT_sb[:, m, :].bitcast(F32), ALU.mult)
                    for nb in range(NSH // 512):
                        nc.tensor.matmul(
                            drow_ps[nb][:], ones_sb[:],
                            sqd[:, nb * 512:(nb + 1) * 512],
                            start=(m == 0), stop=(m == ME - 1))
                for nb in range(NSH // 512):
                    nc.vector.tensor_copy(
                        drow_sb[:, nb * 512:(nb + 1) * 512], drow_ps[nb][:])

            # ============== phase 2/3: gathered passes ==============
            with (
                tc.tile_pool(name="stream", bufs=1) as stream,
                tc.tile_pool(name="work", bufs=1) as work,
                tc.tile_pool(name="psG", bufs=4, space="PSUM") as psG,
            ):
                # Dcol for all shards -> [128, 64]
                for s in range(NCORES):
                    base = s * SH_ELEMS
                    nc.sync.dma_start(
                        dcol_sb[:, s * TSH:(s + 1) * TSH],
                        ag_out[:][base + XP_ELEMS + XPT_ELEMS:base + SH_ELEMS]
                        .rearrange("(p m) -> p m", p=P))

                for sub in range(NSUBS):
                    gwf_ps = [
                        psG.tile([P, EMB], mybir.dt.float32, tag="gwf",
                                 name=f"gwf{e}")
                        for e in range(ME)
                    ]
                    for t in range(TALL):
                        s, lt = t // TSH, t % TSH
                        base = s * SH_ELEMS
                        # stream xpT block (4 nx-tiles worth) and xp tile
                        if t % 4 == 0:
                            lb = lt // 4
                            xpT_blk = stream.tile([P, ME, 512], F32R, tag="xpTb",
                                                  bufs=3, name="xpT_blk")
                            nc.sync.dma_start(
                                xpT_blk[:],
                                ag_out[:][base + XP_ELEMS:base + XP_ELEMS + XPT_ELEMS]
                                .rearrange("(p m n) -> p m n", p=P, m=ME)
                                [:, :, lb * 512:(lb + 1) * 512].bitcast(F32R))
                        xp_t = stream.tile([P, EMB], F32R, tag="xpt", bufs=4,
                                           name="xp_t")
                        nc.sync.dma_start(
                            xp_t[:],
                            ag_out[:][base:base + XP_ELEMS]
                            .rearrange("(p m e) -> p m e", p=P, m=TSH)
                            [:, lt, :].bitcast(F32R))

                        aps = psA.tile([P, NYSUB], mybir.dt.float32, tag="mm",
                                       name="aps")
                        for k in range(ME):
                            nc.tensor.matmul(
                                aps[:], xpT_blk[:, k, (t % 4) * P:(t % 4 + 1) * P],
                                ypT_sb[:, k, sub * NYSUB:(sub + 1) * NYSUB],
                                start=(k == 0), stop=(k == ME - 1))
                        d = work.tile([P, NYSUB], F32, tag="d", bufs=2, name="d")
                        nc.scalar.activation(
                            d[:], drow_sb[:, sub * NYSUB:(sub + 1) * NYSUB],
                            ACTF.Identity, bias=dcol_sb[:, t:t + 1], scale=1.0)
                        r = work.tile([P, NYSUB], F32, tag="r", bufs=2, name="r")
                        nc.vector.reciprocal_approx_fast(out=r[:], in_=d[:])
                        a_sb = work.tile([P, NYSUB], F32R, tag="a", bufs=3,
                                         name="a_sb")
                        nc.vector.scalar_tensor_tensor(
                            out=a_sb[:], in0=aps[:], scalar=2.0, in1=r[:],
                            op0=ALU.mult, op1=ALU.mult)
                        for e in range(ME):
                            nc.tensor.matmul(
                                gwf_ps[e][:], xp_t[:, e * P:(e + 1) * P], a_sb[:],
                                start=(t == 0), stop=(t == TALL - 1))

                    # fused ReLU MLP on gwfT
                    gwfT = work.tile([P, ME, EMB], F32R, tag="gwfT", bufs=1,
                                     name="gwfT")
                    for e in range(ME):
                        nc.vector.tensor_copy(gwfT[:, e, :], gwf_ps[e][:])
                    for m in range(EMB_OUT // P):
                        ps2 = psA.tile([P, NYSUB], mybir.dt.float32, tag="mm",
                                       name="ps_mlp")
                        for k in range(ME):
                            nc.tensor.matmul(
                                ps2[:], Wg_sb[:, k, m * P:(m + 1) * P],
                                gwfT[:, k, :], start=(k == 0), stop=(k == ME - 1))
                        ot = work.tile([P, NYSUB], F32, tag="ot", bufs=2,
                                       name="ot")
                        nc.scalar.activation(
                            ot[:], ps2[:], ACTF.Relu, bias=bgp_sb[:, m:m + 1],
                            scale=1.0)
                        nc.sync.dma_start(
                            outT.ap()[m * P:(m + 1) * P,
                                      sub * NYSUB:(sub + 1) * NYSUB],
                            ot[:])
    nc.compile()
    return nc


def _get_runner():
    """Compile once and return f(concat_inputs_list) -> concat_outputs_list."""
    if "runner" in _CACHE:
        return _CACHE["runner"]

    import jax
    import concourse.mybir as mybir
    from concourse import bass2jax
    from concourse.bass2jax import _bass_exec_p, install_neuronx_cc_hook
    from jax.experimental.shard_map import shard_map
    from jax.sharding import Mesh, PartitionSpec

    nc = _build_nc()
    install_neuronx_cc_hook()

    partition_name = (nc.partition_id_tensor.name
                      if nc.partition_id_tensor else None)
    in_names, out_names, out_avals = [], [], []
    for alloc in nc.m.functions[0].allocations:
        if not isinstance(alloc, mybir.MemoryLocationSet):
            continue
        name = alloc.memorylocations[0].name
        if alloc.kind == "ExternalInput":
            if name != partition_name:
                in_names.append(name)
        elif alloc.kind == "ExternalOutput":
            out_names.append(name)
            out_avals.append(jax.core.ShapedArray(
                tuple(alloc.tensor_shape), mybir.dt.np(alloc.dtype)))
    n_params = len(in_names)
    n_outs = len(out_names)
    all_names = in_names + out_names
    if partition_name is not None:
        all_names = all_names + [partition_name]

    def _body(*args):
        operands = list(args)
        if partition_name is not None:
            operands.append(bass2jax.partition_id_tensor())
        outs = _bass_exec_p.bind(
            *operands,
            out_avals=tuple(out_avals),
            in_names=tuple(all_names),
            out_names=tuple(out_names),
            lowering_input_output_aliases=(),
            sim_require_finite=True,
            sim_require_nnan=True,
            nc=nc,
        )
        return tuple(outs)

    devices = jax.devices()[:NCORES]
    mesh = Mesh(np.asarray(devices), ("core",))
    specs = (PartitionSpec("core"),) * (n_params + n_outs)
    donate = tuple(range(n_params, n_params + n_outs))
    sharded = jax.jit(
        shard_map(_body, mesh=mesh, in_specs=specs,
                  out_specs=(PartitionSpec("core"),) * n_outs, check_rep=False),
        donate_argnums=donate, keep_unused=True,
    )
    out_shapes = [tuple(a.shape) for a in out_avals]
    out_dtypes = [a.dtype for a in out_avals]
    runner = {
        "f": sharded, "in_names": in_names, "out_names": out_names,
        "out_shapes": out_shapes, "out_dtypes": out_dtypes,
    }
    _CACHE["runner"] = runner
    return runner


def _host_prep(x, y, Wx, bx, Wy, by, Wg, bg):
    """Build the concatenated (8*dim0, ...) global input arrays."""
    x = np.ascontiguousarray(x, dtype=np.float32)
    y = np.ascontiguousarray(y, dtype=np.float32)
    xT = x.T  # [FX, NX]
    yT = y.T
    bx_bc = np.tile(np.asarray(bx, np.float32)[None, :], (P, 1))
    bxp = np.asarray(bx, np.float32).reshape(ME, P).T.copy()
    byp = np.asarray(by, np.float32).reshape(ME, P).T.copy()
    bgp = np.asarray(bg, np.float32).reshape(EMB_OUT // P, P).T.copy()
    ones = np.ones((P, P), np.float32)

    per_core = {
        "xT": [np.ascontiguousarray(xT[:, c * NSH:(c + 1) * NSH])
               for c in range(NCORES)],
        "yT": [np.ascontiguousarray(yT[:, c * NSH:(c + 1) * NSH])
               for c in range(NCORES)],
        "Wx": [np.asarray(Wx, np.float32)] * NCORES,
        "Wy": [np.asarray(Wy, np.float32)] * NCORES,
        "Wg": [np.asarray(Wg, np.float32)] * NCORES,
        "bx_bc": [bx_bc] * NCORES,
        "bxp": [bxp] * NCORES,
        "byp": [byp] * NCORES,
        "bgp": [bgp] * NCORES,
        "ones": [ones] * NCORES,
    }
    runner = _get_runner()
    concat = [np.concatenate(per_core[name], axis=0) for name in runner["in_names"]]
    zeros = [np.zeros((NCORES * s[0],) + s[1:], d)
             for s, d in zip(runner["out_shapes"], runner["out_dtypes"])]
    return concat, zeros


def _run(concat, zeros):
    runner = _get_runner()
    out_arrs = runner["f"](*concat, *zeros)
    return out_arrs


def kernel(x, y, Wx, bx, Wy, by, Wg, bg):
    concat, zeros = _host_prep(x, y, Wx, bx, Wy, by, Wg, bg)
    out_arrs = _run(concat, zeros)
    runner = _get_runner()
    idx = runner["out_names"].index("outT")
    outT_all = np.asarray(out_arrs[idx]).reshape(NCORES, EMB_OUT, NSH)
    out = np.empty((NY, EMB_OUT), np.float32)
    for c in range(NCORES):
        out[c * NSH:(c + 1) * NSH, :] = outT_all[c].T
    return out


# revision 21
# speedup vs baseline: 2747.6353x; 2747.6353x over previous
"""Trainium2 Bass kernel for nn_BaseQVLayer (GNN message passing).

Reference computation (single device):
    xp = x @ Wx + bx                      # [Nx, E]
    yp = y @ Wy + by                      # [Ny, E]
    A_ = xp @ yp.T                        # [Nx, Ny]
    A  = 2*A_ / (||xp_i||^2 + ||yp_j||^2) # Dice-style normalization
    gwf = A.T @ xp                        # [Ny, E]
    out = relu(gwf @ Wg + bg)             # [Ny, E]

Distribution: column-parallel over Ny (8 shards of 1024 y-rows, one per
NeuronCore).  Each core needs the *full* xp in two layouts (normal for the
gwf contraction, transposed for the A matmul) plus its own ypT shard.  The
xp/xpT/|xp|^2 pieces are computed per-core for its own Nx shard only and
exchanged with a single packed AllGather; everything downstream is local to
the core, so there is no AllReduce at all.

MM_MODE selects the matmul operand dtype: "bf16" (fast PE path, ~2e-3
relative error) or "f32r" (4-byte rounded mode, ~2.4e-4 error but measured
~5x slower per matmul on hardware).  The normalization chain (Dcol/Drow/
reciprocal) stays fp32 in both modes; in bf16 mode the fp32 Dcol crosses
the AllGather as a bf16 hi/lo split pair.

Per-core phases:
  1. project own shards: xp_shard, xpT_shard, Dcol_shard, ypT_shard, Drow
     (k-major over arriving input slabs to hide the initial DMA stream)
  2. packed AllGather of (xp, xpT, Dcol) shards
  3. shard-rotated passes: each core starts its A/gwf accumulation on its
     own SBUF-resident shard (hiding the AllGather), then walks the other
     7 shards via partition_id-offset reads of the gathered buffer.
     Two ny-sub passes (512 each): A tiles -> fast-reciprocal Dice
     normalization -> gwfT accumulation in PSUM -> fused ReLU-MLP.

kernel(**inputs) takes full unsharded inputs and returns the full output.
"""

import sys

if "/opt/trn_rl_repo" not in sys.path:
    sys.path.insert(0, "/opt/trn_rl_repo")

import numpy as np

MM_MODE = "bf16"   # "bf16" | "f32r"

NCORES = 8
NX, NY = 8192, 8192
FX, FY = 1024, 1024
EMB, EMB_OUT = 512, 512

P = 128
KT = FX // P           # 8   k-tiles over feature dim
ME = EMB // P          # 4   emb tiles
NSH = NX // NCORES     # 1024 rows per shard
TSH = NSH // P         # 8   nx tiles per shard
TALL = NX // P         # 64  nx tiles total
NYSUB = 512            # ny columns per pass
NSUBS = NSH // NYSUB   # 2   passes

XP_ELEMS = P * TSH * EMB          # 524288
XPT_ELEMS = P * ME * NSH          # 524288
DCOL_SLOTS = 2 * P * TSH          # 2048 (hi+lo in bf16 mode; f32 uses half)
SH_ELEMS = XP_ELEMS + XPT_ELEMS + DCOL_SLOTS

_CACHE = {}


def _build_nc(with_collective=True, passes_repeat=1, mm_mode=None):
    import concourse.bass as bass
    from concourse import bacc
    import concourse.mybir as mybir
    import concourse.tile as tile

    mm_mode = mm_mode or MM_MODE
    F32 = mybir.dt.float32
    if mm_mode == "bf16":
        MMD = mybir.dt.bfloat16
        IND = mybir.dt.bfloat16

        def ind(ap):   # DRAM input ap viewed as matmul dtype
            return ap

        def eng(ap):   # matmul-dtype tile viewed for DVE/ACT reads
            return ap
    else:
        MMD = mybir.dt.float32r
        IND = mybir.dt.float32

        def ind(ap):
            return ap.bitcast(mybir.dt.float32r)

        def eng(ap):
            return ap.bitcast(mybir.dt.float32)

    ALU = mybir.AluOpType
    ACTF = mybir.ActivationFunctionType

    nc = bacc.Bacc("TRN2", target_bir_lowering=False, debug=False,
                   num_devices=NCORES if with_collective else 1)

    xT = nc.dram_tensor("xT", [FX, NSH], IND, kind="ExternalInput")
    yT = nc.dram_tensor("yT", [FY, NSH], IND, kind="ExternalInput")
    Wx = nc.dram_tensor("Wx", [FX, EMB], IND, kind="ExternalInput")
    Wy = nc.dram_tensor("Wy", [FY, EMB], IND, kind="ExternalInput")
    Wg = nc.dram_tensor("Wg", [EMB, EMB_OUT], IND, kind="ExternalInput")
    bx_bc = nc.dram_tensor("bx_bc", [P, EMB], F32, kind="ExternalInput")
    bxp = nc.dram_tensor("bxp", [P, ME], F32, kind="ExternalInput")
    byp = nc.dram_tensor("byp", [P, ME], F32, kind="ExternalInput")
    bgp = nc.dram_tensor("bgp", [P, EMB_OUT // P], F32, kind="ExternalInput")
    ones = nc.dram_tensor("ones", [P, P], IND, kind="ExternalInput")
    outT = nc.dram_tensor("outT", [EMB_OUT, NSH], F32, kind="ExternalOutput")

    with tile.TileContext(nc) as tc:
        with (
            tc.tile_pool(name="perm", bufs=1) as perm,
            tc.tile_pool(name="psA", bufs=3, space="PSUM") as psA,
            tc.tile_pool(name="dramp", bufs=1, space="DRAM") as dramp,
        ):
            # ---- permanent tiles ----
            ypT_sb = perm.tile([P, ME, NSH], MMD)
            drow_sb = perm.tile([P, NSH], F32)
            dcol_rot = perm.tile([P, TALL], F32)
            Wg_sb = perm.tile([P, ME, EMB_OUT], MMD)
            bgp_sb = perm.tile([P, EMB_OUT // P], F32)
            # own-shard projections stay resident so pass A/G can start on
            # them before the AllGather completes (shard-rotated t order)
            xp_sb = perm.tile([P, TSH, EMB], MMD)
            xpT_sb = perm.tile([P, ME, NSH], MMD)
            dcol_own = perm.tile([P, TSH], F32)
            nc.sync.dma_start(
                Wg_sb[:], ind(Wg.ap().rearrange("(kt p) n -> p kt n", p=P)))
            nc.sync.dma_start(bgp_sb[:], bgp.ap())

            ag_in = dramp.tile([SH_ELEMS], MMD)
            ag_out = dramp.tile([NCORES * SH_ELEMS], MMD, addr_space="Shared")

            # ================= phase 1: own-shard projections ================
            with (
                tc.tile_pool(name="wpool", bufs=1) as wpool,
                tc.tile_pool(name="scr", bufs=2) as scr,
                tc.tile_pool(name="ph1ps", bufs=2, space="PSUM") as ph1ps,
            ):
                xT_sb = wpool.tile([P, KT, NSH], MMD)
                yT_sb = wpool.tile([P, KT, NSH], MMD)
                Wx_sb = wpool.tile([P, KT, EMB], MMD)
                Wy_sb = wpool.tile([P, KT, EMB], MMD)
                bx_bc_sb = wpool.tile([P, EMB], F32)
                bxp_sb = wpool.tile([P, ME], F32)
                byp_sb = wpool.tile([P, ME], F32)
                ones_sb = wpool.tile([P, P], MMD)
                for k in range(KT):
                    nc.sync.dma_start(
                        Wx_sb[:, k, :], ind(Wx.ap()[k * P:(k + 1) * P, :]))
                    nc.sync.dma_start(
                        xT_sb[:, k, :], ind(xT.ap()[k * P:(k + 1) * P, :]))
                    nc.sync.dma_start(
                        Wy_sb[:, k, :], ind(Wy.ap()[k * P:(k + 1) * P, :]))
                    nc.sync.dma_start(
                        yT_sb[:, k, :], ind(yT.ap()[k * P:(k + 1) * P, :]))
                nc.sync.dma_start(bx_bc_sb[:], bx_bc.ap())
                nc.sync.dma_start(bxp_sb[:], bxp.ap())
                nc.sync.dma_start(byp_sb[:], byp.ap())
                nc.sync.dma_start(ones_sb[:], ind(ones.ap()))

                # xp shard: [128, m, 512], nx on partitions.  k-major across
                # all 8 m-groups (8 concurrent PSUM banks) so PE issues 8
                # matmuls per arriving xT k-slab instead of stalling on the
                # full xT stream.
                ap = ag_in[:]
                xp_region = ap[0:XP_ELEMS].rearrange(
                    "(p m e) -> p m e", p=P, m=TSH)
                xpT_region = ap[XP_ELEMS:XP_ELEMS + XPT_ELEMS].rearrange(
                    "(p m n) -> p m n", p=P, m=ME)
                xp_grp = []
                for m in range(TSH):
                    pool_m = psA if m < 3 else ph1ps
                    tag_m = "mm" if m < 3 else "grp"
                    xp_grp.append(pool_m.tile(
                        [P, EMB], mybir.dt.float32, tag=tag_m,
                        bufs=(3 if m < 3 else 5),
                        name=f"ps_xp{m}"))
                for k in range(KT):
                    for m in range(TSH):
                        nc.tensor.matmul(
                            xp_grp[m][:], xT_sb[:, k, m * P:(m + 1) * P],
                            Wx_sb[:, k, :],
                            start=(k == 0), stop=(k == KT - 1))
                for m in range(TSH):
                    nc.vector.tensor_tensor(
                        xp_sb[:, m, :], xp_grp[m][:], bx_bc_sb[:], ALU.add)
                    sq = scr.tile([P, EMB], F32, tag="sq", name="sq")
                    nc.scalar.activation(
                        sq[:], eng(xp_sb[:, m, :]), ACTF.Square,
                        scale=1.0, accum_out=dcol_own[:, m:m + 1])

                # xpT shard: [128, me, 1024], emb on partitions
                for m in range(ME):
                    for nb in range(NSH // 512):
                        ps = psA.tile([P, 512], mybir.dt.float32, tag="mm",
                                      name="ps_xpt")
                        for k in range(KT):
                            nc.tensor.matmul(
                                ps[:], Wx_sb[:, k, m * P:(m + 1) * P],
                                xT_sb[:, k, nb * 512:(nb + 1) * 512],
                                start=(k == 0), stop=(k == KT - 1))
                        nc.scalar.activation(
                            xpT_sb[:, m, nb * 512:(nb + 1) * 512], ps[:],
                            ACTF.Identity, bias=bxp_sb[:, m:m + 1], scale=1.0)

                # pack ag_in: xp, xpT, and Dcol (hi/lo split when bf16)
                for m in range(TSH):
                    nc.sync.dma_start(xp_region[:, m, :], xp_sb[:, m, :])
                nc.sync.dma_start(xpT_region[:], xpT_sb[:])
                dc_region = ap[XP_ELEMS + XPT_ELEMS:SH_ELEMS].rearrange(
                    "(h p m) -> h p m", h=2, p=P)
                if mm_mode == "bf16":
                    dc_hi = scr.tile([P, TSH], MMD, tag="dchi", name="dc_hi")
                    dc_lo = scr.tile([P, TSH], MMD, tag="dclo", name="dc_lo")
                    nc.vector.tensor_copy(dc_hi[:], dcol_own[:])
                    nc.vector.tensor_tensor(
                        dc_lo[:], dcol_own[:], dc_hi[:], ALU.subtract)
                    nc.sync.dma_start(dc_region[0], dc_hi[:])
                    nc.sync.dma_start(dc_region[1], dc_lo[:])
                else:
                    nc.sync.dma_start(
                        ap[XP_ELEMS + XPT_ELEMS:XP_ELEMS + XPT_ELEMS
                           + P * TSH * 2].bitcast(F32)
                        .rearrange("(p m) -> p m", p=P),
                        dcol_own[:])
                if with_collective:
                    nc.gpsimd.collective_compute(
                        "AllGather", ALU.bypass,
                        replica_groups=[list(range(NCORES))],
                        ins=[ag_in[:].opt()],
                        outs=[ag_out[:].opt()],
                    )

                # ypT shard (overlaps the AllGather).  nb-outer order so the
                # sub=0 half (ypT columns 0:512 + Drow 0:512) completes first
                # and pass A can start early.  Drow = colsum(ypT^2)
                # broadcast to all partitions via ones-matmul.
                for nb in range(NSH // 512):
                    drow_ps = ph1ps.tile([P, 512], mybir.dt.float32, tag="grp",
                                         bufs=5, name=f"drow_ps{nb}")
                    for m in range(ME):
                        ps = psA.tile([P, 512], mybir.dt.float32, tag="mm",
                                      name="ps_ypt")
                        for k in range(KT):
                            nc.tensor.matmul(
                                ps[:], Wy_sb[:, k, m * P:(m + 1) * P],
                                yT_sb[:, k, nb * 512:(nb + 1) * 512],
                                start=(k == 0), stop=(k == KT - 1))
                        nc.scalar.activation(
                            ypT_sb[:, m, nb * 512:(nb + 1) * 512], ps[:],
                            ACTF.Identity, bias=byp_sb[:, m:m + 1], scale=1.0)
                        sqd = scr.tile([P, 512], MMD, tag="sqd", name="sqd")
                        nc.vector.tensor_tensor(
                            sqd[:], eng(ypT_sb[:, m, nb * 512:(nb + 1) * 512]),
                            eng(ypT_sb[:, m, nb * 512:(nb + 1) * 512]),
                            ALU.mult)
                        nc.tensor.matmul(
                            drow_ps[:], ones_sb[:], sqd[:],
                            start=(m == 0), stop=(m == ME - 1))
                    nc.vector.tensor_copy(
                        drow_sb[:, nb * 512:(nb + 1) * 512], drow_ps[:])

            # ============== phase 2/3: gathered passes ==============
            with (
                tc.tile_pool(name="stream", bufs=1) as stream,
                tc.tile_pool(name="work", bufs=1) as work,
                tc.tile_pool(name="psG", bufs=4, space="PSUM") as psG,
            ):
                # shard-rotation: core c processes shard order
                # c, c+1, ..., c+7 (mod 8).  j=0 reads its own projections
                # straight from SBUF (no AllGather dependency); j>=1 reads
                # the gathered buffer at a partition_id-dependent offset, by
                # which time the AllGather has completed behind phase-1 work.
                import concourse.bass as bass_mod
                pid = nc.sync.partition_id() if with_collective else 0
                bases = [None] + [
                    ((pid + j) % NCORES) * SH_ELEMS for j in range(1, NCORES)
                ]
                # Dcol for rotated shards j>=1 -> dcol_rot[:, j*8:(j+1)*8]
                for j in range(1, NCORES):
                    dcap = ag_out[:][bass_mod.ds(
                        bases[j] + XP_ELEMS + XPT_ELEMS, DCOL_SLOTS)]
                    if mm_mode == "bf16":
                        dc2 = dcap.rearrange("(h p m) -> h p m", h=2, p=P)
                        dch = scr2 = stream.tile([P, TSH], MMD, tag="dch",
                                                 bufs=2, name="dch")
                        dcl = stream.tile([P, TSH], MMD, tag="dcl", bufs=2,
                                          name="dcl")
                        nc.sync.dma_start(dch[:], dc2[0])
                        nc.sync.dma_start(dcl[:], dc2[1])
                        nc.vector.tensor_tensor(
                            dcol_rot[:, j * TSH:(j + 1) * TSH],
                            dch[:], dcl[:], ALU.add)
                    else:
                        nc.sync.dma_start(
                            dcol_rot[:, j * TSH:(j + 1) * TSH],
                            dcap[0:P * TSH * 2].bitcast(F32)
                            .rearrange("(p m) -> p m", p=P))

                for sub in [s for _ in range(passes_repeat)
                            for s in range(NSUBS)]:
                    gwf_ps = [
                        psG.tile([P, EMB], mybir.dt.float32, tag="gwf",
                                 name=f"gwf{e}")
                        for e in range(ME)
                    ]
                    # software pipeline: gwf matmuls for iteration t are
                    # emitted after the A matmuls of t+1, so PE always has
                    # independent work while DVE produces a_sb(t).
                    pending = None  # (xp_lhs, a_sb, t)

                    def flush_gwf():
                        nonlocal pending
                        if pending is None:
                            return
                        xp_l, a_l, tl = pending
                        for e in range(ME):
                            nc.tensor.matmul(
                                gwf_ps[e][:], xp_l[:, e * P:(e + 1) * P],
                                a_l[:],
                                start=(tl == 0), stop=(tl == TALL - 1))
                        pending = None

                    for t in range(TALL):
                        j, lt = t // TSH, t % TSH
                        if j == 0:
                            xpT_lhs = xpT_sb
                            xp_lhs = xp_sb[:, lt, :]
                            dcol_bias = dcol_own[:, lt:lt + 1]
                            xpT_col = lt * P
                        else:
                            # stream xpT block (4 nx-tiles) and xp tile
                            if t % 4 == 0:
                                lb = lt // 4
                                xpT_blk = stream.tile(
                                    [P, ME, 512], MMD, tag="xpTb", bufs=3,
                                    name="xpT_blk")
                                nc.sync.dma_start(
                                    xpT_blk[:],
                                    ag_out[:][bass_mod.ds(
                                        bases[j] + XP_ELEMS, XPT_ELEMS)]
                                    .rearrange("(p m n) -> p m n", p=P, m=ME)
                                    [:, :, lb * 512:(lb + 1) * 512])
                            xp_t = stream.tile([P, EMB], MMD, tag="xpt",
                                               bufs=4, name="xp_t")
                            nc.sync.dma_start(
                                xp_t[:],
                                ag_out[:][bass_mod.ds(bases[j], XP_ELEMS)]
                                .rearrange("(p m e) -> p m e", p=P, m=TSH)
                                [:, lt, :])
                            xpT_lhs = xpT_blk
                            xp_lhs = xp_t[:]
                            dcol_bias = dcol_rot[:, t:t + 1]
                            xpT_col = (t % 4) * P

                        aps = psA.tile([P, NYSUB], mybir.dt.float32, tag="mm",
                                       name="aps")
                        for k in range(ME):
                            nc.tensor.matmul(
                                aps[:], xpT_lhs[:, k, xpT_col:xpT_col + P],
                                ypT_sb[:, k, sub * NYSUB:(sub + 1) * NYSUB],
                                start=(k == 0), stop=(k == ME - 1))
                        flush_gwf()
                        d = work.tile([P, NYSUB], F32, tag="d", bufs=3,
                                      name="d")
                        nc.scalar.activation(
                            d[:], drow_sb[:, sub * NYSUB:(sub + 1) * NYSUB],
                            ACTF.Identity, bias=dcol_bias, scale=1.0)
                        r = work.tile([P, NYSUB], F32, tag="r", bufs=3,
                                      name="r")
                        nc.vector.reciprocal_approx_fast(out=r[:], in_=d[:])
                        a_sb = work.tile([P, NYSUB], MMD, tag="a", bufs=4,
                                         name="a_sb")
                        nc.vector.scalar_tensor_tensor(
                            out=a_sb[:], in0=aps[:], scalar=2.0, in1=r[:],
                            op0=ALU.mult, op1=ALU.mult)
                        pending = (xp_lhs, a_sb, t)
                    flush_gwf()

                    # fused ReLU MLP on gwfT
                    gwfT = work.tile([P, ME, EMB], MMD, tag="gwfT", bufs=1,
                                     name="gwfT")
                    for e in range(ME):
                        nc.vector.tensor_copy(gwfT[:, e, :], gwf_ps[e][:])
                    for m in range(EMB_OUT // P):
                        ps2 = psA.tile([P, NYSUB], mybir.dt.float32, tag="mm",
                                       name="ps_mlp")
                        for k in range(ME):
                            nc.tensor.matmul(
                                ps2[:], Wg_sb[:, k, m * P:(m + 1) * P],
                                gwfT[:, k, :], start=(k == 0),
                                stop=(k == ME - 1))
                        ot = work.tile([P, NYSUB], F32, tag="ot", bufs=2,
                                       name="ot")
                        nc.scalar.activation(
                            ot[:], ps2[:], ACTF.Relu, bias=bgp_sb[:, m:m + 1],
                            scale=1.0)
                        nc.sync.dma_start(
                            outT.ap()[m * P:(m + 1) * P,
                                      sub * NYSUB:(sub + 1) * NYSUB],
                            ot[:])
    nc.compile()
    return nc


def _get_runner():
    """Compile once and return the jitted 8-core runner + metadata."""
    if "runner" in _CACHE:
        return _CACHE["runner"]

    import jax
    import concourse.mybir as mybir
    from concourse import bass2jax
    from concourse.bass2jax import _bass_exec_p, install_neuronx_cc_hook
    from jax.experimental.shard_map import shard_map
    from jax.sharding import Mesh, PartitionSpec

    nc = _build_nc()
    install_neuronx_cc_hook()

    partition_name = (nc.partition_id_tensor.name
                      if nc.partition_id_tensor else None)
    in_names, out_names, out_avals = [], [], []
    for alloc in nc.m.functions[0].allocations:
        if not isinstance(alloc, mybir.MemoryLocationSet):
            continue
        name = alloc.memorylocations[0].name
        if alloc.kind == "ExternalInput":
            if name != partition_name:
                in_names.append(name)
        elif alloc.kind == "ExternalOutput":
            out_names.append(name)
            out_avals.append(jax.core.ShapedArray(
                tuple(alloc.tensor_shape), mybir.dt.np(alloc.dtype)))
    n_params = len(in_names)
    n_outs = len(out_names)
    all_names = in_names + out_names
    if partition_name is not None:
        all_names = all_names + [partition_name]

    def _body(*args):
        operands = list(args)
        if partition_name is not None:
            operands.append(bass2jax.partition_id_tensor())
        outs = _bass_exec_p.bind(
            *operands,
            out_avals=tuple(out_avals),
            in_names=tuple(all_names),
            out_names=tuple(out_names),
            lowering_input_output_aliases=(),
            sim_require_finite=True,
            sim_require_nnan=True,
            nc=nc,
        )
        return tuple(outs)

    devices = jax.devices()[:NCORES]
    mesh = Mesh(np.asarray(devices), ("core",))
    specs = (PartitionSpec("core"),) * (n_params + n_outs)
    donate = tuple(range(n_params, n_params + n_outs))
    sharded = jax.jit(
        shard_map(_body, mesh=mesh, in_specs=specs,
                  out_specs=(PartitionSpec("core"),) * n_outs, check_rep=False),
        donate_argnums=donate, keep_unused=True,
    )
    runner = {
        "f": sharded, "in_names": in_names, "out_names": out_names,
        "out_shapes": [tuple(a.shape) for a in out_avals],
        "out_dtypes": [a.dtype for a in out_avals],
    }
    _CACHE["runner"] = runner
    return runner


def _host_prep(x, y, Wx, bx, Wy, by, Wg, bg):
    """Build the concatenated (8*dim0, ...) global input arrays."""
    import ml_dtypes

    in_dt = ml_dtypes.bfloat16 if MM_MODE == "bf16" else np.float32
    x = np.ascontiguousarray(x, dtype=np.float32)
    y = np.ascontiguousarray(y, dtype=np.float32)
    xT = x.T.astype(in_dt)  # [FX, NX]
    yT = y.T.astype(in_dt)
    bx_bc = np.tile(np.asarray(bx, np.float32)[None, :], (P, 1))
    bxp = np.asarray(bx, np.float32).reshape(ME, P).T.copy()
    byp = np.asarray(by, np.float32).reshape(ME, P).T.copy()
    bgp = np.asarray(bg, np.float32).reshape(EMB_OUT // P, P).T.copy()
    ones = np.ones((P, P), in_dt)

    per_core = {
        "xT": [np.ascontiguousarray(xT[:, c * NSH:(c + 1) * NSH])
               for c in range(NCORES)],
        "yT": [np.ascontiguousarray(yT[:, c * NSH:(c + 1) * NSH])
               for c in range(NCORES)],
        "Wx": [np.asarray(Wx, np.float32).astype(in_dt)] * NCORES,
        "Wy": [np.asarray(Wy, np.float32).astype(in_dt)] * NCORES,
        "Wg": [np.asarray(Wg, np.float32).astype(in_dt)] * NCORES,
        "bx_bc": [bx_bc] * NCORES,
        "bxp": [bxp] * NCORES,
        "byp": [byp] * NCORES,
        "bgp": [bgp] * NCORES,
        "ones": [ones] * NCORES,
    }
    runner = _get_runner()
    concat = [np.concatenate(per_core[name], axis=0)
              for name in runner["in_names"]]
    zeros = [np.zeros((NCORES * s[0],) + s[1:], d)
             for s, d in zip(runner["out_shapes"], runner["out_dtypes"])]
    return concat, zeros


def kernel(x, y, Wx, bx, Wy, by, Wg, bg):
    concat, zeros = _host_prep(x, y, Wx, bx, Wy, by, Wg, bg)
    runner = _get_runner()
    out_arrs = runner["f"](*concat, *zeros)
    idx = runner["out_names"].index("outT")
    outT_all = np.asarray(out_arrs[idx]).reshape(NCORES, EMB_OUT, NSH)
    out = np.empty((NY, EMB_OUT), np.float32)
    for c in range(NCORES):
        out[c * NSH:(c + 1) * NSH, :] = outT_all[c].T
    return out
